# revision 22
# baseline (speedup 1.0000x reference)
"""CondConv (routing -> per-sample mixed 3x3 conv -> frozen BN -> ReLU -> residual)
on 8 Trainium2 NeuronCores, data-parallel over batch (4 samples/core).

Per core:
  - full expert bank resident in SBUF, host-pretransposed to [cin, kk, cout]
  - routing: GAP (DVE reduce) -> dot with route_w (DVE + gpsimd partition
    all-reduce; keeps the PE queue free for conv matmuls) -> sigmoid (ACT)
  - per-sample mixed kernel: 4x DVE scalar_tensor_tensor accumulation
  - conv: per output tile, 18 accumulating float32r matmuls (2 cin tiles x 3x3
    taps) against a zero-padded [128, 58, 58] input image; moving dim = 7 rows
    x 56 cols = 392 (>=256 keeps fp32r at 1 cycle/row)
  - BN(frozen)+ReLU fused into the ACT PSUM evacuation, residual add on DVE
"""

import threading

import numpy as np

import concourse.bass as bass
import concourse.mybir as mybir
import concourse.tile as tile
from concourse import bacc, bass_isa
from concourse.bass_utils import run_bass_kernel_spmd

F32 = mybir.dt.float32
F32R = mybir.dt.float32r
AX = mybir.AxisListType
OP = mybir.AluOpType
AF = mybir.ActivationFunctionType

N_CORES = 8
B, CIN, COUT, H, W, KS, E = 32, 256, 256, 56, 56, 3, 4
BPC = B // N_CORES  # samples per core
CT = CIN // 128     # cin partition tiles
OTN = COUT // 128   # cout partition tiles
KK = KS * KS
WP = W + 2          # width zero-padded (kj shifts); height handled by clipping
RC = 8              # row chunks per image
RH = H // RC        # rows per chunk
NF = RH * W         # moving-dim elements per matmul
BN_EPS = 1e-5

# conv taps, center first: the center tap covers the full output chunk, so it
# carries start=True and clears every PSUM has_written bit; row-clipped taps
# then accumulate flat sub-slices (= 'same' padding semantics at top/bottom).
TAPS = [(1, 1)] + [(ki, kj) for ki in range(KS) for kj in range(KS)
                   if (ki, kj) != (1, 1)]


def build_bass():
    nc = bacc.Bacc("TRN2", target_bir_lowering=False, debug=False)

    x_d = nc.dram_tensor("x", [BPC, CIN, H, W], F32, kind="ExternalInput")
    wt_d = nc.dram_tensor("wt", [E, CIN, KK, COUT], F32, kind="ExternalInput")
    rwt_d = nc.dram_tensor("rwt", [CIN, E], F32, kind="ExternalInput")
    rb_d = nc.dram_tensor("rb", [E], F32, kind="ExternalInput")
    bnp_d = nc.dram_tensor("bnp", [4, COUT], F32, kind="ExternalInput")
    y_d = nc.dram_tensor("y", [BPC, COUT, H, W], F32, kind="ExternalOutput")

    x_ap = x_d.ap()
    wt_ap = wt_d.ap()
    rwt_ap = rwt_d.ap()
    rb_ap = rb_d.ap()
    bnp_ap = bnp_d.ap()
    y_ap = y_d.ap()

    with tile.TileContext(nc) as tc:
        with (
            tc.tile_pool(name="wbp", bufs=1) as wbp,
            tc.tile_pool(name="xpp", bufs=1) as xpp,
            tc.tile_pool(name="mwp", bufs=1) as mwp,
            tc.tile_pool(name="otp", bufs=10) as otp,
            tc.tile_pool(name="snp", bufs=1) as snp,
            tc.tile_pool(name="smp", bufs=4) as smp,
            tc.tile_pool(name="psp", bufs=6, space="PSUM") as psp,
        ):
            # ---- persistent tiles ----
            wb = [[wbp.tile([128, KK, COUT], F32, name=f"wb{e}_{t}", tag=f"wb{e}_{t}")
                   for t in range(CT)] for e in range(E)]
            xp = [[xpp.tile([128, H, WP], F32, name=f"xp{i}_{t}", tag=f"xp{i}_{t}")
                   for t in range(CT)] for i in range(2)]
            mw = [[mwp.tile([128, KK, COUT], F32, name=f"mw{i}_{t}", tag=f"mw{i}_{t}")
                   for t in range(CT)] for i in range(2)]
            rwt_sb = [snp.tile([128, E], F32, name=f"rwt{t}", tag=f"rwt{t}")
                      for t in range(CT)]
            rb_bc = snp.tile([128, E], F32, name="rb_bc", tag="rb_bc")
            bn_sb = [snp.tile([128, 4], F32, name=f"bn{o}", tag=f"bn{o}")
                     for o in range(OTN)]
            bn_inv = [snp.tile([128, 1], F32, name=f"bninv{o}", tag=f"bninv{o}")
                      for o in range(OTN)]
            bn_shift = [snp.tile([128, 1], F32, name=f"bnsh{o}", tag=f"bnsh{o}")
                        for o in range(OTN)]

            # ---- preamble: input DMAs (x on sync queue; the rest on ACT queue) ----
            for t in range(CT):
                nc.sync.dma_start(out=xp[0][t][:, :, 1:W + 1].bitcast(F32R),
                                  in_=x_ap[0, t * 128:(t + 1) * 128, :, :].bitcast(F32R))
            for t in range(CT):
                nc.scalar.dma_start(out=rwt_sb[t],
                                    in_=rwt_ap[t * 128:(t + 1) * 128, :])
            nc.scalar.dma_start(
                out=rb_bc,
                in_=bass.AP(tensor=rb_ap.tensor, offset=0, ap=[[0, 128], [1, E]]))
            for o in range(OTN):
                nc.scalar.dma_start(
                    out=bn_sb[o],
                    in_=bnp_ap[:, o * 128:(o + 1) * 128].transpose([1, 0]))
            for e in range(E):
                for t in range(CT):
                    nc.scalar.dma_start(out=wb[e][t],
                                        in_=wt_ap[e, t * 128:(t + 1) * 128, :, :])

            # all-zeros per-partition scalar: explicit AP bias for ACT funcs
            # (the float-bias path needs a pre-registered const-AP database)
            zeros1 = snp.tile([128, 1], F32, name="zeros1", tag="zeros1")
            nc.vector.memset(zeros1, 0.0)

            # zero the width-pad border columns once via DMA from a zeroed
            # strip (gpsimd memset can't emit float32r, DMA bit-copy can)
            zcol = snp.tile([128, H], F32, name="zcol", tag="zcol")
            nc.vector.memset(zcol, 0.0)
            for i in range(2):
                for t in range(CT):
                    nc.scalar.dma_start(out=xp[i][t][:, :, 0].bitcast(F32R),
                                        in_=zcol.bitcast(F32R))
                    nc.scalar.dma_start(out=xp[i][t][:, :, WP - 1].bitcast(F32R),
                                        in_=zcol.bitcast(F32R))

            # BN folded scale/shift: inv = gamma / sqrt(var+eps);
            # shift = beta - mean * inv
            for o in range(OTN):
                ve = smp.tile([128, 1], F32, name=f"ve{o}", tag="ve")
                nc.vector.tensor_scalar_add(ve, bn_sb[o][:, 3:4], BN_EPS)
                sq = smp.tile([128, 1], F32, name=f"sq{o}", tag="sq")
                nc.scalar.activation(out=sq, in_=ve, func=AF.Sqrt, bias=zeros1)
                nc.vector.reciprocal(out=bn_inv[o], in_=sq)
                nc.vector.tensor_mul(bn_inv[o], bn_inv[o], bn_sb[o][:, 0:1])
                mi = smp.tile([128, 1], F32, name=f"mi{o}", tag="mi")
                nc.vector.tensor_mul(mi, bn_sb[o][:, 2:3], bn_inv[o])
                nc.vector.tensor_sub(bn_shift[o], bn_sb[o][:, 1:2], mi)

            def prep(s):
                """Routing + weight mixing for sample s (no PE involvement)."""
                i = s % 2
                pooled = [smp.tile([128, 1], F32, name=f"pool{s}_{t}", tag=f"pool{t}")
                          for t in range(CT)]
                for t in range(CT):
                    nc.vector.reduce_sum(out=pooled[t], in_=xp[i][t][:, :, 1:W + 1],
                                         axis=AX.XY)
                prod = smp.tile([128, E], F32, name=f"prod{s}", tag="prod")
                nc.vector.tensor_scalar_mul(prod, rwt_sb[0], pooled[0])
                nc.vector.scalar_tensor_tensor(out=prod, in0=rwt_sb[1],
                                               scalar=pooled[1], in1=prod,
                                               op0=OP.mult, op1=OP.add)
                lg = smp.tile([128, E], F32, name=f"lg{s}", tag="lg")
                nc.gpsimd.partition_all_reduce(lg, prod, channels=128,
                                               reduce_op=bass_isa.ReduceOp.add)
                nc.vector.scalar_tensor_tensor(out=lg, in0=lg,
                                               scalar=1.0 / (H * W), in1=rb_bc,
                                               op0=OP.mult, op1=OP.add)
                rr = smp.tile([128, E], F32, name=f"rr{s}", tag="rr")
                nc.scalar.activation(out=rr, in_=lg, func=AF.Sigmoid, bias=zeros1)
                for t in range(CT):
                    nc.vector.tensor_scalar_mul(mw[i][t].bitcast(F32R),
                                                wb[0][t], rr[:, 0:1])
                    for e in range(1, E):
                        nc.vector.scalar_tensor_tensor(
                            out=mw[i][t].bitcast(F32R), in0=wb[e][t],
                            scalar=rr[:, e:e + 1],
                            in1=mw[i][t], op0=OP.mult, op1=OP.add)

            def conv(s, oi):
                """One output channel tile of sample s: matmuls + BN/ReLU +
                residual + store."""
                i = s % 2
                o0 = oi * 128
                n_mm = len(TAPS) * CT
                for rc in range(RC):
                    r0 = rc * RH
                    acc = psp.tile([128, NF], F32, name=f"acc{s}_{oi}_{rc}",
                                   tag="acc")
                    k = 0
                    for ki, kj in TAPS:
                        # valid output rows for this tap (clipped at top/
                        # bottom; kj handled by the zero-padded columns)
                        h_lo = max(r0, 1 - ki)
                        h_hi = min(r0 + RH - 1, H - ki)
                        kki = ki * KS + kj
                        for t in range(CT):
                            nc.tensor.matmul(
                                acc[:, (h_lo - r0) * W:(h_hi - r0 + 1) * W],
                                lhsT=mw[i][t][:, kki, o0:o0 + 128].bitcast(F32R),
                                rhs=xp[i][t][:, h_lo + ki - 1:h_hi + ki,
                                             kj:kj + W].bitcast(F32R),
                                start=(k == 0), stop=(k == n_mm - 1))
                            k += 1
                    ob = otp.tile([128, NF], F32, name=f"ob{s}_{oi}_{rc}",
                                  tag="ob")
                    nc.scalar.activation(out=ob[:], in_=acc[:], func=AF.Relu,
                                         bias=bn_shift[oi], scale=bn_inv[oi])
                    ob3 = ob.rearrange("p (a b) -> p a b", a=RH)
                    nc.vector.tensor_add(ob3, ob3,
                                         xp[i][oi][:, r0:r0 + RH, 1:W + 1])
                    nc.sync.dma_start(out=y_ap[s, o0:o0 + 128, r0:r0 + RH, :],
                                      in_=ob3)

            prep(0)
            for s in range(BPC):
                if s + 1 < BPC:
                    j = (s + 1) % 2
                    for t in range(CT):
                        nc.sync.dma_start(
                            out=xp[j][t][:, :, 1:W + 1].bitcast(F32R),
                            in_=x_ap[s + 1, t * 128:(t + 1) * 128, :, :].bitcast(F32R))
                conv(s, 0)
                if s + 1 < BPC:
                    prep(s + 1)
                conv(s, 1)

    nc.compile()
    return nc


_CACHE = {}
_LOCK = threading.Lock()


def _get_nc():
    with _LOCK:
        if "nc" not in _CACHE:
            _CACHE["nc"] = build_bass()
        return _CACHE["nc"]


def prepare_in_maps(inputs):
    """Host-side layout prep (sharding + transposes only)."""
    x = np.ascontiguousarray(np.asarray(inputs["x"], dtype=np.float32))
    route_w = np.asarray(inputs["route_w"], dtype=np.float32)
    route_b = np.ascontiguousarray(np.asarray(inputs["route_b"], dtype=np.float32))
    expert_w = np.asarray(inputs["expert_w"], dtype=np.float32)
    bn_gamma = np.asarray(inputs["bn_gamma"], dtype=np.float32)
    bn_beta = np.asarray(inputs["bn_beta"], dtype=np.float32)
    bn_mean = np.asarray(inputs["bn_mean"], dtype=np.float32)
    bn_var = np.asarray(inputs["bn_var"], dtype=np.float32)

    wt = np.ascontiguousarray(
        expert_w.transpose(0, 2, 3, 4, 1)).reshape(E, CIN, KK, COUT)
    rwt = np.ascontiguousarray(route_w.T)  # [CIN, E]
    bnp = np.ascontiguousarray(
        np.stack([bn_gamma, bn_beta, bn_mean, bn_var], axis=0))  # [4, COUT]

    return [
        {"x": np.ascontiguousarray(x[c * BPC:(c + 1) * BPC]),
         "wt": wt, "rwt": rwt, "rb": route_b, "bnp": bnp}
        for c in range(N_CORES)
    ]


def kernel(**inputs):
    in_maps = prepare_in_maps(inputs)
    nc = _get_nc()
    res = run_bass_kernel_spmd(nc, in_maps, core_ids=list(range(N_CORES)))
    return np.concatenate([r["y"] for r in res.results], axis=0)


# revision 28
# speedup vs baseline: 1.1772x; 1.1772x over previous
"""CondConv (routing -> per-sample mixed 3x3 conv -> frozen BN -> ReLU -> residual)
on 8 Trainium2 NeuronCores, data-parallel over batch (4 samples/core).

Per core:
  - full expert bank resident in SBUF, host-pretransposed to [cin, kk, cout]
  - routing: GAP (DVE reduce) -> dot with route_w (DVE + gpsimd partition
    all-reduce; keeps the PE queue free for conv matmuls) -> sigmoid (ACT)
  - per-sample mixed kernel: 4x DVE scalar_tensor_tensor accumulation
  - conv: per output tile, 18 accumulating float32r matmuls (2 cin tiles x 3x3
    taps) against a zero-padded [128, 58, 58] input image; moving dim = 7 rows
    x 56 cols = 392 (>=256 keeps fp32r at 1 cycle/row)
  - BN(frozen)+ReLU fused into the ACT PSUM evacuation, residual add on DVE
"""

import threading

import numpy as np

import concourse.bass as bass
import concourse.mybir as mybir
import concourse.tile as tile
from concourse import bacc, bass_isa
from concourse.bass_utils import run_bass_kernel_spmd

F32 = mybir.dt.float32
F32R = mybir.dt.float32r
AX = mybir.AxisListType
OP = mybir.AluOpType
AF = mybir.ActivationFunctionType

N_CORES = 8
B, CIN, COUT, H, W, KS, E = 32, 256, 256, 56, 56, 3, 4
BPC = B // N_CORES  # samples per core
CT = CIN // 128     # cin partition tiles
OTN = COUT // 128   # cout partition tiles
KK = KS * KS
WP = W + 2          # width zero-padded (kj shifts); height handled by clipping
RC = 7              # row chunks per image
RH = H // RC        # rows per chunk
NF = RH * W         # moving-dim elements per matmul (448: matches the ~187ns
                    # fp32 weight-load so LDW fully hides under the stream)
BN_EPS = 1e-5

# conv taps, center first: the center tap covers the full output chunk, so it
# carries start=True and clears every PSUM has_written bit; row-clipped taps
# then accumulate flat sub-slices (= 'same' padding semantics at top/bottom).
TAPS = [(1, 1)] + [(ki, kj) for ki in range(KS) for kj in range(KS)
                   if (ki, kj) != (1, 1)]


def build_bass():
    nc = bacc.Bacc("TRN2", target_bir_lowering=False, debug=False)

    x_d = nc.dram_tensor("x", [BPC, CIN, H, WP], F32, kind="ExternalInput")
    wt_d = nc.dram_tensor("wt", [E, CIN, KK, COUT], F32, kind="ExternalInput")
    rwt_d = nc.dram_tensor("rwt", [CIN, E], F32, kind="ExternalInput")
    rb_d = nc.dram_tensor("rb", [E], F32, kind="ExternalInput")
    bnp_d = nc.dram_tensor("bnp", [COUT, 4], F32, kind="ExternalInput")
    y_d = nc.dram_tensor("y", [BPC, COUT, H, W], F32, kind="ExternalOutput")

    x_ap = x_d.ap()
    wt_ap = wt_d.ap()
    rwt_ap = rwt_d.ap()
    rb_ap = rb_d.ap()
    bnp_ap = bnp_d.ap()
    y_ap = y_d.ap()

    with tile.TileContext(nc) as tc:
        with (
            tc.tile_pool(name="wbp", bufs=1) as wbp,
            tc.tile_pool(name="xpp", bufs=1) as xpp,
            tc.tile_pool(name="mwp", bufs=1) as mwp,
            tc.tile_pool(name="otp", bufs=10) as otp,
            tc.tile_pool(name="snp", bufs=1) as snp,
            tc.tile_pool(name="smp", bufs=4) as smp,
            tc.tile_pool(name="psp", bufs=6, space="PSUM") as psp,
        ):
            # ---- persistent tiles ----
            wb = [[wbp.tile([128, KK, COUT], F32, name=f"wb{e}_{t}", tag=f"wb{e}_{t}")
                   for t in range(CT)] for e in range(E)]
            xp = [[xpp.tile([128, H, WP], F32, name=f"xp{i}_{t}", tag=f"xp{i}_{t}")
                   for t in range(CT)] for i in range(2)]
            mw = [[mwp.tile([128, KK, COUT], F32, name=f"mw{i}_{t}", tag=f"mw{i}_{t}")
                   for t in range(CT)] for i in range(2)]
            rwt_sb = [snp.tile([128, E], F32, name=f"rwt{t}", tag=f"rwt{t}")
                      for t in range(CT)]
            rb_bc = snp.tile([128, E], F32, name="rb_bc", tag="rb_bc")
            bn_sb = [snp.tile([128, 4], F32, name=f"bn{o}", tag=f"bn{o}")
                     for o in range(OTN)]
            bn_inv = [snp.tile([128, 1], F32, name=f"bninv{o}", tag=f"bninv{o}")
                      for o in range(OTN)]
            bn_shift = [snp.tile([128, 1], F32, name=f"bnsh{o}", tag=f"bnsh{o}")
                        for o in range(OTN)]

            # ---- preamble: input DMAs (x on sync queue; the rest on ACT queue) ----
            for t in range(CT):
                nc.sync.dma_start(out=xp[0][t].bitcast(F32R),
                                  in_=x_ap[0, t * 128:(t + 1) * 128, :, :].bitcast(F32R))
            for t in range(CT):
                nc.scalar.dma_start(out=rwt_sb[t],
                                    in_=rwt_ap[t * 128:(t + 1) * 128, :])
            nc.scalar.dma_start(
                out=rb_bc,
                in_=bass.AP(tensor=rb_ap.tensor, offset=0, ap=[[0, 128], [1, E]]))
            for o in range(OTN):
                nc.scalar.dma_start(out=bn_sb[o],
                                    in_=bnp_ap[o * 128:(o + 1) * 128, :])
            # expert bank on two queues so the 9.4MB streams in parallel
            for e in range(E):
                eng = nc.scalar if e < 2 else nc.gpsimd
                for t in range(CT):
                    eng.dma_start(out=wb[e][t],
                                  in_=wt_ap[e, t * 128:(t + 1) * 128, :, :])

            # all-zeros per-partition scalar: explicit AP bias for ACT funcs
            # (the float-bias path needs a pre-registered const-AP database)
            zeros1 = snp.tile([128, 1], F32, name="zeros1", tag="zeros1")
            nc.vector.memset(zeros1, 0.0)

            # BN folded scale/shift: inv = gamma / sqrt(var+eps);
            # shift = beta - mean * inv
            for o in range(OTN):
                ve = smp.tile([128, 1], F32, name=f"ve{o}", tag="ve")
                nc.vector.tensor_scalar_add(ve, bn_sb[o][:, 3:4], BN_EPS)
                sq = smp.tile([128, 1], F32, name=f"sq{o}", tag="sq")
                nc.scalar.activation(out=sq, in_=ve, func=AF.Sqrt, bias=zeros1)
                nc.vector.reciprocal(out=bn_inv[o], in_=sq)
                nc.vector.tensor_mul(bn_inv[o], bn_inv[o], bn_sb[o][:, 0:1])
                mi = smp.tile([128, 1], F32, name=f"mi{o}", tag="mi")
                nc.vector.tensor_mul(mi, bn_sb[o][:, 2:3], bn_inv[o])
                nc.vector.tensor_sub(bn_shift[o], bn_sb[o][:, 1:2], mi)

            def prep(s):
                """Routing + weight mixing for sample s (no PE involvement)."""
                i = s % 2
                pooled = [smp.tile([128, 1], F32, name=f"pool{s}_{t}", tag=f"pool{t}")
                          for t in range(CT)]
                for t in range(CT):
                    nc.vector.reduce_sum(out=pooled[t], in_=xp[i][t][:, :, 1:W + 1],
                                         axis=AX.XY)
                prod = smp.tile([128, E], F32, name=f"prod{s}", tag="prod")
                nc.vector.tensor_scalar_mul(prod, rwt_sb[0], pooled[0])
                nc.vector.scalar_tensor_tensor(out=prod, in0=rwt_sb[1],
                                               scalar=pooled[1], in1=prod,
                                               op0=OP.mult, op1=OP.add)
                lg = smp.tile([128, E], F32, name=f"lg{s}", tag="lg")
                nc.gpsimd.partition_all_reduce(lg, prod, channels=128,
                                               reduce_op=bass_isa.ReduceOp.add)
                nc.vector.scalar_tensor_tensor(out=lg, in0=lg,
                                               scalar=1.0 / (H * W), in1=rb_bc,
                                               op0=OP.mult, op1=OP.add)
                rr = smp.tile([128, E], F32, name=f"rr{s}", tag="rr")
                nc.scalar.activation(out=rr, in_=lg, func=AF.Sigmoid, bias=zeros1)
                for t in range(CT):
                    nc.vector.tensor_scalar_mul(mw[i][t].bitcast(F32R),
                                                wb[0][t], rr[:, 0:1])
                    for e in range(1, E):
                        nc.vector.scalar_tensor_tensor(
                            out=mw[i][t].bitcast(F32R), in0=wb[e][t],
                            scalar=rr[:, e:e + 1],
                            in1=mw[i][t], op0=OP.mult, op1=OP.add)

            def conv(s, oi):
                """One output channel tile of sample s: matmuls + BN/ReLU +
                residual + store."""
                i = s % 2
                o0 = oi * 128
                n_mm = len(TAPS) * CT
                for rc in range(RC):
                    r0 = rc * RH
                    acc = psp.tile([128, NF], F32, name=f"acc{s}_{oi}_{rc}",
                                   tag="acc")
                    k = 0
                    for ki, kj in TAPS:
                        # valid output rows for this tap (clipped at top/
                        # bottom; kj handled by the zero-padded columns)
                        h_lo = max(r0, 1 - ki)
                        h_hi = min(r0 + RH - 1, H - ki)
                        kki = ki * KS + kj
                        for t in range(CT):
                            nc.tensor.matmul(
                                acc[:, (h_lo - r0) * W:(h_hi - r0 + 1) * W],
                                lhsT=mw[i][t][:, kki, o0:o0 + 128].bitcast(F32R),
                                rhs=xp[i][t][:, h_lo + ki - 1:h_hi + ki,
                                             kj:kj + W].bitcast(F32R),
                                start=(k == 0), stop=(k == n_mm - 1))
                            k += 1
                    ob = otp.tile([128, NF], F32, name=f"ob{s}_{oi}_{rc}",
                                  tag="ob")
                    nc.scalar.activation(out=ob[:], in_=acc[:], func=AF.Relu,
                                         bias=bn_shift[oi], scale=bn_inv[oi])
                    ob3 = ob.rearrange("p (a b) -> p a b", a=RH)
                    nc.vector.tensor_add(ob3, ob3,
                                         xp[i][oi][:, r0:r0 + RH, 1:W + 1])
                    nc.sync.dma_start(out=y_ap[s, o0:o0 + 128, r0:r0 + RH, :],
                                      in_=ob3)

            prep(0)
            for s in range(BPC):
                if s + 1 < BPC:
                    j = (s + 1) % 2
                    for t in range(CT):
                        nc.sync.dma_start(
                            out=xp[j][t].bitcast(F32R),
                            in_=x_ap[s + 1, t * 128:(t + 1) * 128, :, :].bitcast(F32R))
                conv(s, 0)
                if s + 1 < BPC:
                    prep(s + 1)
                conv(s, 1)

    nc.compile()
    return nc


_CACHE = {}
_LOCK = threading.Lock()


def _get_nc():
    with _LOCK:
        if "nc" not in _CACHE:
            _CACHE["nc"] = build_bass()
        return _CACHE["nc"]


def prepare_in_maps(inputs):
    """Host-side layout prep (sharding + transposes only)."""
    x = np.ascontiguousarray(np.asarray(inputs["x"], dtype=np.float32))
    route_w = np.asarray(inputs["route_w"], dtype=np.float32)
    route_b = np.ascontiguousarray(np.asarray(inputs["route_b"], dtype=np.float32))
    expert_w = np.asarray(inputs["expert_w"], dtype=np.float32)
    bn_gamma = np.asarray(inputs["bn_gamma"], dtype=np.float32)
    bn_beta = np.asarray(inputs["bn_beta"], dtype=np.float32)
    bn_mean = np.asarray(inputs["bn_mean"], dtype=np.float32)
    bn_var = np.asarray(inputs["bn_var"], dtype=np.float32)

    wt = np.ascontiguousarray(
        expert_w.transpose(0, 2, 3, 4, 1)).reshape(E, CIN, KK, COUT)
    rwt = np.ascontiguousarray(route_w.T)  # [CIN, E]
    bnp = np.ascontiguousarray(
        np.stack([bn_gamma, bn_beta, bn_mean, bn_var], axis=1))  # [COUT, 4]

    # width-pad on host: border columns arrive pre-zeroed, so the device DMA
    # is one fully contiguous transfer per (sample, cin-tile)
    xpad = np.zeros((B, CIN, H, WP), dtype=np.float32)
    xpad[:, :, :, 1:W + 1] = x

    return [
        {"x": np.ascontiguousarray(xpad[c * BPC:(c + 1) * BPC]),
         "wt": wt, "rwt": rwt, "rb": route_b, "bnp": bnp}
        for c in range(N_CORES)
    ]


def kernel(**inputs):
    in_maps = prepare_in_maps(inputs)
    nc = _get_nc()
    res = run_bass_kernel_spmd(nc, in_maps, core_ids=list(range(N_CORES)))
    return np.concatenate([r["y"] for r in res.results], axis=0)


# revision 29
# speedup vs baseline: 1.3404x; 1.1387x over previous
"""CondConv (routing -> per-sample mixed 3x3 conv -> frozen BN -> ReLU -> residual)
on 8 Trainium2 NeuronCores, data-parallel over batch (4 samples/core).

Per core:
  - expert bank resident in SBUF as bf16, host-pretransposed to
    [cin, cout-half, kk, 128] so each cout half is contiguous
  - routing: GAP (DVE reduce) -> dot with route_w (DVE + gpsimd partition
    all-reduce; keeps the PE queue free for conv matmuls) -> sigmoid (ACT)
  - per-sample mixed kernel: DVE scalar_tensor_tensor accumulation in bf16,
    split per cout half so the first conv starts after half the mixing
  - conv: per output tile, 18 accumulating bf16 matmuls (2 cin tiles x 3x3
    taps; fp32 PSUM) against width-padded bf16 images; moving dim = 8 rows
    x 56 cols = 448; bf16 weight loads get FWL so LDW hides under the stream
  - BN(frozen)+ReLU fused into the ACT PSUM evacuation, residual add on DVE,
    fp32 output
"""

import threading

import ml_dtypes
import numpy as np

import concourse.bass as bass
import concourse.mybir as mybir
import concourse.tile as tile
from concourse import bacc, bass_isa
from concourse.bass_utils import run_bass_kernel_spmd

F32 = mybir.dt.float32
BF16 = mybir.dt.bfloat16
AX = mybir.AxisListType
OP = mybir.AluOpType
AF = mybir.ActivationFunctionType

N_CORES = 8
B, CIN, COUT, H, W, KS, E = 32, 256, 256, 56, 56, 3, 4
BPC = B // N_CORES  # samples per core
CT = CIN // 128     # cin partition tiles
OTN = COUT // 128   # cout partition tiles
KK = KS * KS
WP = W + 2          # width zero-padded (kj shifts); height handled by clipping
RC = 7              # row chunks per image
RH = H // RC        # rows per chunk
NF = RH * W         # moving-dim elements per matmul
BN_EPS = 1e-5

# conv taps, center first: the center tap covers the full output chunk, so it
# carries start=True and clears every PSUM has_written bit; row-clipped taps
# then accumulate flat sub-slices (= 'same' padding semantics at top/bottom).
TAPS = [(1, 1)] + [(ki, kj) for ki in range(KS) for kj in range(KS)
                   if (ki, kj) != (1, 1)]


def build_bass():
    nc = bacc.Bacc("TRN2", target_bir_lowering=False, debug=False)

    x_d = nc.dram_tensor("x", [BPC, CIN, H, WP], BF16, kind="ExternalInput")
    wt_d = nc.dram_tensor("wt", [E, CIN, OTN, KK, 128], BF16,
                          kind="ExternalInput")
    rwt_d = nc.dram_tensor("rwt", [CIN, E], F32, kind="ExternalInput")
    rb_d = nc.dram_tensor("rb", [E], F32, kind="ExternalInput")
    bnp_d = nc.dram_tensor("bnp", [COUT, 4], F32, kind="ExternalInput")
    y_d = nc.dram_tensor("y", [BPC, COUT, H, W], F32, kind="ExternalOutput")

    x_ap = x_d.ap()
    wt_ap = wt_d.ap()
    rwt_ap = rwt_d.ap()
    rb_ap = rb_d.ap()
    bnp_ap = bnp_d.ap()
    y_ap = y_d.ap()

    with tile.TileContext(nc) as tc:
        with (
            tc.tile_pool(name="wbp", bufs=1) as wbp,
            tc.tile_pool(name="xpp", bufs=1) as xpp,
            tc.tile_pool(name="mwp", bufs=1) as mwp,
            tc.tile_pool(name="otp", bufs=10) as otp,
            tc.tile_pool(name="snp", bufs=1) as snp,
            tc.tile_pool(name="smp", bufs=4) as smp,
            tc.tile_pool(name="psp", bufs=6, space="PSUM") as psp,
        ):
            # ---- persistent tiles ----
            wb = [[wbp.tile([128, OTN, KK, 128], BF16, name=f"wb{e}_{t}",
                            tag=f"wb{e}_{t}")
                   for t in range(CT)] for e in range(E)]
            xp = [[xpp.tile([128, H, WP], BF16, name=f"xp{i}_{t}",
                            tag=f"xp{i}_{t}")
                   for t in range(CT)] for i in range(2)]
            mw = [[mwp.tile([128, OTN, KK, 128], BF16, name=f"mw{i}_{t}",
                            tag=f"mw{i}_{t}")
                   for t in range(CT)] for i in range(2)]
            rwt_sb = [snp.tile([128, E], F32, name=f"rwt{t}", tag=f"rwt{t}")
                      for t in range(CT)]
            rb_bc = snp.tile([128, E], F32, name="rb_bc", tag="rb_bc")
            bn_sb = [snp.tile([128, 4], F32, name=f"bn{o}", tag=f"bn{o}")
                     for o in range(OTN)]
            bn_inv = [snp.tile([128, 1], F32, name=f"bninv{o}", tag=f"bninv{o}")
                      for o in range(OTN)]
            bn_shift = [snp.tile([128, 1], F32, name=f"bnsh{o}", tag=f"bnsh{o}")
                        for o in range(OTN)]

            # ---- preamble DMAs: x on sync queue, expert bank split across
            # the scalar and gpsimd queues, small params on scalar ----
            for t in range(CT):
                nc.sync.dma_start(out=xp[0][t],
                                  in_=x_ap[0, t * 128:(t + 1) * 128, :, :])
            for t in range(CT):
                nc.scalar.dma_start(out=rwt_sb[t],
                                    in_=rwt_ap[t * 128:(t + 1) * 128, :])
            nc.scalar.dma_start(
                out=rb_bc,
                in_=bass.AP(tensor=rb_ap.tensor, offset=0, ap=[[0, 128], [1, E]]))
            for o in range(OTN):
                nc.scalar.dma_start(out=bn_sb[o],
                                    in_=bnp_ap[o * 128:(o + 1) * 128, :])
            for e in range(E):
                eng = nc.scalar if e < 2 else nc.gpsimd
                for t in range(CT):
                    eng.dma_start(out=wb[e][t],
                                  in_=wt_ap[e, t * 128:(t + 1) * 128])

            # all-zeros per-partition scalar: explicit AP bias for ACT funcs
            # (the float-bias path needs a pre-registered const-AP database)
            zeros1 = snp.tile([128, 1], F32, name="zeros1", tag="zeros1")
            nc.vector.memset(zeros1, 0.0)

            # BN folded scale/shift: inv = gamma / sqrt(var+eps);
            # shift = beta - mean * inv
            for o in range(OTN):
                ve = smp.tile([128, 1], F32, name=f"ve{o}", tag="ve")
                nc.vector.tensor_scalar_add(ve, bn_sb[o][:, 3:4], BN_EPS)
                sq = smp.tile([128, 1], F32, name=f"sq{o}", tag="sq")
                nc.scalar.activation(out=sq, in_=ve, func=AF.Sqrt, bias=zeros1)
                nc.vector.reciprocal(out=bn_inv[o], in_=sq)
                nc.vector.tensor_mul(bn_inv[o], bn_inv[o], bn_sb[o][:, 0:1])
                mi = smp.tile([128, 1], F32, name=f"mi{o}", tag="mi")
                nc.vector.tensor_mul(mi, bn_sb[o][:, 2:3], bn_inv[o])
                nc.vector.tensor_sub(bn_shift[o], bn_sb[o][:, 1:2], mi)

            def prep(s):
                """Routing + weight mixing for sample s (no PE involvement)."""
                i = s % 2
                pooled = [smp.tile([128, 1], F32, name=f"pool{s}_{t}",
                                   tag=f"pool{t}") for t in range(CT)]
                for t in range(CT):
                    nc.vector.reduce_sum(out=pooled[t],
                                         in_=xp[i][t][:, :, 1:W + 1],
                                         axis=AX.XY)
                prod = smp.tile([128, E], F32, name=f"prod{s}", tag="prod")
                nc.vector.tensor_scalar_mul(prod, rwt_sb[0], pooled[0])
                nc.vector.scalar_tensor_tensor(out=prod, in0=rwt_sb[1],
                                               scalar=pooled[1], in1=prod,
                                               op0=OP.mult, op1=OP.add)
                lg = smp.tile([128, E], F32, name=f"lg{s}", tag="lg")
                nc.gpsimd.partition_all_reduce(lg, prod, channels=128,
                                               reduce_op=bass_isa.ReduceOp.add)
                nc.vector.scalar_tensor_tensor(out=lg, in0=lg,
                                               scalar=1.0 / (H * W), in1=rb_bc,
                                               op0=OP.mult, op1=OP.add)
                rr = smp.tile([128, E], F32, name=f"rr{s}", tag="rr")
                nc.scalar.activation(out=rr, in_=lg, func=AF.Sigmoid, bias=zeros1)
                # mix per cout half: the first conv of the sample only waits
                # for the oi=0 half of the bank
                for oi in range(OTN):
                    for t in range(CT):
                        nc.vector.tensor_scalar_mul(mw[i][t][:, oi],
                                                    wb[0][t][:, oi], rr[:, 0:1])
                        for e in range(1, E):
                            nc.vector.scalar_tensor_tensor(
                                out=mw[i][t][:, oi], in0=wb[e][t][:, oi],
                                scalar=rr[:, e:e + 1], in1=mw[i][t][:, oi],
                                op0=OP.mult, op1=OP.add)

            def conv(s, oi):
                """One output channel tile of sample s: matmuls + BN/ReLU +
                residual + store."""
                i = s % 2
                o0 = oi * 128
                n_mm = len(TAPS) * CT
                for rc in range(RC):
                    r0 = rc * RH
                    acc = psp.tile([128, NF], F32, name=f"acc{s}_{oi}_{rc}",
                                   tag="acc")
                    k = 0
                    for ki, kj in TAPS:
                        # valid output rows for this tap (clipped at top/
                        # bottom; kj handled by the zero-padded columns)
                        h_lo = max(r0, 1 - ki)
                        h_hi = min(r0 + RH - 1, H - ki)
                        kki = ki * KS + kj
                        for t in range(CT):
                            nc.tensor.matmul(
                                acc[:, (h_lo - r0) * W:(h_hi - r0 + 1) * W],
                                lhsT=mw[i][t][:, oi, kki, :],
                                rhs=xp[i][t][:, h_lo + ki - 1:h_hi + ki,
                                             kj:kj + W],
                                start=(k == 0), stop=(k == n_mm - 1))
                            k += 1
                    ob = otp.tile([128, NF], F32, name=f"ob{s}_{oi}_{rc}",
                                  tag="ob")
                    nc.scalar.activation(out=ob[:], in_=acc[:], func=AF.Relu,
                                         bias=bn_shift[oi], scale=bn_inv[oi])
                    ob3 = ob.rearrange("p (a b) -> p a b", a=RH)
                    nc.vector.tensor_add(ob3, ob3,
                                         xp[i][oi][:, r0:r0 + RH, 1:W + 1])
                    nc.sync.dma_start(out=y_ap[s, o0:o0 + 128, r0:r0 + RH, :],
                                      in_=ob3)

            prep(0)
            for s in range(BPC):
                if s + 1 < BPC:
                    j = (s + 1) % 2
                    for t in range(CT):
                        nc.sync.dma_start(
                            out=xp[j][t],
                            in_=x_ap[s + 1, t * 128:(t + 1) * 128, :, :])
                conv(s, 0)
                if s + 1 < BPC:
                    prep(s + 1)
                conv(s, 1)

    nc.compile()
    return nc


_CACHE = {}
_LOCK = threading.Lock()


def prepare_in_maps(inputs):
    """Host-side layout prep (sharding + transposes + dtype casts only)."""
    x = np.asarray(inputs["x"], dtype=np.float32)
    route_w = np.asarray(inputs["route_w"], dtype=np.float32)
    route_b = np.ascontiguousarray(np.asarray(inputs["route_b"], dtype=np.float32))
    expert_w = np.asarray(inputs["expert_w"], dtype=np.float32)
    bn_gamma = np.asarray(inputs["bn_gamma"], dtype=np.float32)
    bn_beta = np.asarray(inputs["bn_beta"], dtype=np.float32)
    bn_mean = np.asarray(inputs["bn_mean"], dtype=np.float32)
    bn_var = np.asarray(inputs["bn_var"], dtype=np.float32)

    # [E, COUT, CIN, K, K] -> [E, CIN, K, K, COUT] -> [E, CIN, OTN, KK, 128]
    wt = (expert_w.transpose(0, 2, 3, 4, 1)
          .reshape(E, CIN, KK, OTN, 128)
          .transpose(0, 1, 3, 2, 4))
    wt = np.ascontiguousarray(wt).astype(ml_dtypes.bfloat16)
    rwt = np.ascontiguousarray(route_w.T)  # [CIN, E]
    bnp = np.ascontiguousarray(
        np.stack([bn_gamma, bn_beta, bn_mean, bn_var], axis=1))  # [COUT, 4]

    # width-pad on host: border columns arrive pre-zeroed, so the device DMA
    # is one fully contiguous transfer per (sample, cin-tile)
    xpad = np.zeros((B, CIN, H, WP), dtype=ml_dtypes.bfloat16)
    xpad[:, :, :, 1:W + 1] = x.astype(ml_dtypes.bfloat16)

    return [
        {"x": np.ascontiguousarray(xpad[c * BPC:(c + 1) * BPC]),
         "wt": wt, "rwt": rwt, "rb": route_b, "bnp": bnp}
        for c in range(N_CORES)
    ]


def _get_nc():
    with _LOCK:
        if "nc" not in _CACHE:
            _CACHE["nc"] = build_bass()
        return _CACHE["nc"]


def kernel(**inputs):
    in_maps = prepare_in_maps(inputs)
    nc = _get_nc()
    res = run_bass_kernel_spmd(nc, in_maps, core_ids=list(range(N_CORES)))
    return np.concatenate([r["y"] for r in res.results], axis=0)


# revision 32
# speedup vs baseline: 1.4407x; 1.0748x over previous
"""CondConv (routing -> per-sample mixed 3x3 conv -> frozen BN -> ReLU -> residual)
on 8 Trainium2 NeuronCores, data-parallel over batch (4 samples/core).

Per core:
  - expert bank resident in SBUF as bf16, host-pretransposed to
    [cin, cout-half, kk, 128] so each cout half is contiguous
  - routing: GAP (DVE reduce) -> dot with route_w (DVE + gpsimd partition
    all-reduce; keeps the PE queue free for conv matmuls) -> sigmoid (ACT)
  - per-sample mixed kernel: DVE scalar_tensor_tensor accumulation in bf16,
    split per cout half so the first conv starts after half the mixing
  - conv: per output tile, 18 accumulating bf16 matmuls (2 cin tiles x 3x3
    taps; fp32 PSUM) against width-padded bf16 images; moving dim = 8 rows
    x 56 cols = 448; bf16 weight loads get FWL so LDW hides under the stream
  - BN(frozen)+ReLU fused into the ACT PSUM evacuation, residual add on DVE,
    fp32 output
"""

import threading

import ml_dtypes
import numpy as np

import concourse.bass as bass
import concourse.mybir as mybir
import concourse.tile as tile
from concourse import bacc, bass_isa
from concourse.bass_utils import run_bass_kernel_spmd

F32 = mybir.dt.float32
BF16 = mybir.dt.bfloat16
AX = mybir.AxisListType
OP = mybir.AluOpType
AF = mybir.ActivationFunctionType

N_CORES = 8
B, CIN, COUT, H, W, KS, E = 32, 256, 256, 56, 56, 3, 4
BPC = B // N_CORES  # samples per core
CT = CIN // 128     # cin partition tiles
OTN = COUT // 128   # cout partition tiles
KK = KS * KS
WP = W + 2          # width zero-padded (kj shifts); height handled by clipping
RC = 7              # row chunks per image
RH = H // RC        # rows per chunk
NF = RH * W         # moving-dim elements per matmul
BN_EPS = 1e-5

# conv taps, center first: the center tap covers the full output chunk, so it
# carries start=True and clears every PSUM has_written bit; row-clipped taps
# then accumulate flat sub-slices (= 'same' padding semantics at top/bottom).
TAPS = [(1, 1)] + [(ki, kj) for ki in range(KS) for kj in range(KS)
                   if (ki, kj) != (1, 1)]


def build_bass():
    nc = bacc.Bacc("TRN2", target_bir_lowering=False, debug=False)

    x_d = nc.dram_tensor("x", [BPC, CIN, H, WP], BF16, kind="ExternalInput")
    wt_d = nc.dram_tensor("wt", [E, CIN, OTN, KK, 128], BF16,
                          kind="ExternalInput")
    rwt_d = nc.dram_tensor("rwt", [CIN, E], F32, kind="ExternalInput")
    rb_d = nc.dram_tensor("rb", [E], F32, kind="ExternalInput")
    bnp_d = nc.dram_tensor("bnp", [COUT, 4], F32, kind="ExternalInput")
    y_d = nc.dram_tensor("y", [BPC, COUT, H, W], F32, kind="ExternalOutput")

    x_ap = x_d.ap()
    wt_ap = wt_d.ap()
    rwt_ap = rwt_d.ap()
    rb_ap = rb_d.ap()
    bnp_ap = bnp_d.ap()
    y_ap = y_d.ap()

    with tile.TileContext(nc) as tc:
        with (
            tc.tile_pool(name="wbp", bufs=1) as wbp,
            tc.tile_pool(name="xpp", bufs=1) as xpp,
            tc.tile_pool(name="mwp", bufs=1) as mwp,
            tc.tile_pool(name="otp", bufs=10) as otp,
            tc.tile_pool(name="snp", bufs=1) as snp,
            tc.tile_pool(name="smp", bufs=4) as smp,
            tc.tile_pool(name="psp", bufs=6, space="PSUM") as psp,
        ):
            # ---- persistent tiles ----
            wb = [[wbp.tile([128, OTN, KK, 128], BF16, name=f"wb{e}_{t}",
                            tag=f"wb{e}_{t}")
                   for t in range(CT)] for e in range(E)]
            xp = [[xpp.tile([128, H, WP], BF16, name=f"xp{i}_{t}",
                            tag=f"xp{i}_{t}")
                   for t in range(CT)] for i in range(2)]
            mw = [[mwp.tile([128, OTN, KK, 128], BF16, name=f"mw{i}_{t}",
                            tag=f"mw{i}_{t}")
                   for t in range(CT)] for i in range(2)]
            rwt_sb = [snp.tile([128, E], F32, name=f"rwt{t}", tag=f"rwt{t}")
                      for t in range(CT)]
            rb_bc = snp.tile([128, E], F32, name="rb_bc", tag="rb_bc")
            bn_sb = [snp.tile([128, 4], F32, name=f"bn{o}", tag=f"bn{o}")
                     for o in range(OTN)]
            bn_inv = [snp.tile([128, 1], F32, name=f"bninv{o}", tag=f"bninv{o}")
                      for o in range(OTN)]
            bn_shift = [snp.tile([128, 1], F32, name=f"bnsh{o}", tag=f"bnsh{o}")
                        for o in range(OTN)]

            # ---- preamble DMAs, all on the sync queue in priority order:
            # one HWDGE queue saturates HBM, so queue order = bandwidth
            # priority. x(0) first (heads the routing critical path), tiny
            # params next, then the expert bank (needed at mix time, later).
            for t in range(CT):
                nc.sync.dma_start(out=xp[0][t],
                                  in_=x_ap[0, t * 128:(t + 1) * 128, :, :])
            for t in range(CT):
                nc.sync.dma_start(out=rwt_sb[t],
                                  in_=rwt_ap[t * 128:(t + 1) * 128, :])
            nc.sync.dma_start(
                out=rb_bc,
                in_=bass.AP(tensor=rb_ap.tensor, offset=0, ap=[[0, 128], [1, E]]))
            for o in range(OTN):
                nc.sync.dma_start(out=bn_sb[o],
                                  in_=bnp_ap[o * 128:(o + 1) * 128, :])
            for e in range(E):
                for t in range(CT):
                    nc.sync.dma_start(out=wb[e][t],
                                      in_=wt_ap[e, t * 128:(t + 1) * 128])

            # all-zeros per-partition scalar: explicit AP bias for ACT funcs
            # (the float-bias path needs a pre-registered const-AP database)
            zeros1 = snp.tile([128, 1], F32, name="zeros1", tag="zeros1")
            nc.vector.memset(zeros1, 0.0)

            # BN folded scale/shift: inv = gamma / sqrt(var+eps);
            # shift = beta - mean * inv
            for o in range(OTN):
                ve = smp.tile([128, 1], F32, name=f"ve{o}", tag="ve")
                nc.vector.tensor_scalar_add(ve, bn_sb[o][:, 3:4], BN_EPS)
                sq = smp.tile([128, 1], F32, name=f"sq{o}", tag="sq")
                nc.scalar.activation(out=sq, in_=ve, func=AF.Sqrt, bias=zeros1)
                nc.vector.reciprocal(out=bn_inv[o], in_=sq)
                nc.vector.tensor_mul(bn_inv[o], bn_inv[o], bn_sb[o][:, 0:1])
                mi = smp.tile([128, 1], F32, name=f"mi{o}", tag="mi")
                nc.vector.tensor_mul(mi, bn_sb[o][:, 2:3], bn_inv[o])
                nc.vector.tensor_sub(bn_shift[o], bn_sb[o][:, 1:2], mi)

            # scratch target for the ACT-side pooled copy (only accum_out used)
            pscr = snp.tile([128, H * W], BF16, name="pscr", tag="pscr")

            def prep(s):
                """Routing + weight mixing for sample s (no PE involvement)."""
                i = s % 2
                pooled = [smp.tile([128, 1], F32, name=f"pool{s}_{t}",
                                   tag=f"pool{t}") for t in range(CT)]
                # GAP: tile 0 on DVE, tile 1 on ACT (Copy + accum_out) so the
                # two image reductions run in parallel
                nc.vector.reduce_sum(out=pooled[0], in_=xp[i][0][:, :, 1:W + 1],
                                     axis=AX.XY)
                nc.scalar.activation(out=pscr, in_=xp[i][1][:, :, 1:W + 1],
                                     func=AF.Copy, accum_out=pooled[1])
                prod = smp.tile([128, E], F32, name=f"prod{s}", tag="prod")
                nc.vector.tensor_scalar_mul(prod, rwt_sb[0], pooled[0])
                nc.vector.scalar_tensor_tensor(out=prod, in0=rwt_sb[1],
                                               scalar=pooled[1], in1=prod,
                                               op0=OP.mult, op1=OP.add)
                lg = smp.tile([128, E], F32, name=f"lg{s}", tag="lg")
                nc.gpsimd.partition_all_reduce(lg, prod, channels=128,
                                               reduce_op=bass_isa.ReduceOp.add)
                nc.vector.scalar_tensor_tensor(out=lg, in0=lg,
                                               scalar=1.0 / (H * W), in1=rb_bc,
                                               op0=OP.mult, op1=OP.add)
                rr = smp.tile([128, E], F32, name=f"rr{s}", tag="rr")
                nc.scalar.activation(out=rr, in_=lg, func=AF.Sigmoid, bias=zeros1)
                # mix per cout half: the first conv of the sample only waits
                # for the oi=0 half of the bank
                for oi in range(OTN):
                    for t in range(CT):
                        nc.vector.tensor_scalar_mul(mw[i][t][:, oi],
                                                    wb[0][t][:, oi], rr[:, 0:1])
                        for e in range(1, E):
                            nc.vector.scalar_tensor_tensor(
                                out=mw[i][t][:, oi], in0=wb[e][t][:, oi],
                                scalar=rr[:, e:e + 1], in1=mw[i][t][:, oi],
                                op0=OP.mult, op1=OP.add)

            def conv(s, oi):
                """One output channel tile of sample s: matmuls + BN/ReLU +
                residual + store."""
                i = s % 2
                o0 = oi * 128
                n_mm = len(TAPS) * CT
                for rc in range(RC):
                    r0 = rc * RH
                    acc = psp.tile([128, NF], F32, name=f"acc{s}_{oi}_{rc}",
                                   tag="acc")
                    k = 0
                    for ki, kj in TAPS:
                        # valid output rows for this tap (clipped at top/
                        # bottom; kj handled by the zero-padded columns)
                        h_lo = max(r0, 1 - ki)
                        h_hi = min(r0 + RH - 1, H - ki)
                        kki = ki * KS + kj
                        for t in range(CT):
                            nc.tensor.matmul(
                                acc[:, (h_lo - r0) * W:(h_hi - r0 + 1) * W],
                                lhsT=mw[i][t][:, oi, kki, :],
                                rhs=xp[i][t][:, h_lo + ki - 1:h_hi + ki,
                                             kj:kj + W],
                                start=(k == 0), stop=(k == n_mm - 1))
                            k += 1
                    ob = otp.tile([128, NF], F32, name=f"ob{s}_{oi}_{rc}",
                                  tag="ob")
                    nc.scalar.activation(out=ob[:], in_=acc[:], func=AF.Relu,
                                         bias=bn_shift[oi], scale=bn_inv[oi])
                    ob3 = ob.rearrange("p (a b) -> p a b", a=RH)
                    nc.vector.tensor_add(ob3, ob3,
                                         xp[i][oi][:, r0:r0 + RH, 1:W + 1])
                    nc.sync.dma_start(out=y_ap[s, o0:o0 + 128, r0:r0 + RH, :],
                                      in_=ob3)

            prep(0)
            for s in range(BPC):
                if s + 1 < BPC:
                    j = (s + 1) % 2
                    for t in range(CT):
                        nc.sync.dma_start(
                            out=xp[j][t],
                            in_=x_ap[s + 1, t * 128:(t + 1) * 128, :, :])
                conv(s, 0)
                if s + 1 < BPC:
                    prep(s + 1)
                conv(s, 1)

    nc.compile()
    return nc


_CACHE = {}
_LOCK = threading.Lock()


def prepare_in_maps(inputs):
    """Host-side layout prep (sharding + transposes + dtype casts only)."""
    x = np.asarray(inputs["x"], dtype=np.float32)
    route_w = np.asarray(inputs["route_w"], dtype=np.float32)
    route_b = np.ascontiguousarray(np.asarray(inputs["route_b"], dtype=np.float32))
    expert_w = np.asarray(inputs["expert_w"], dtype=np.float32)
    bn_gamma = np.asarray(inputs["bn_gamma"], dtype=np.float32)
    bn_beta = np.asarray(inputs["bn_beta"], dtype=np.float32)
    bn_mean = np.asarray(inputs["bn_mean"], dtype=np.float32)
    bn_var = np.asarray(inputs["bn_var"], dtype=np.float32)

    # [E, COUT, CIN, K, K] -> [E, CIN, K, K, COUT] -> [E, CIN, OTN, KK, 128]
    wt = (expert_w.transpose(0, 2, 3, 4, 1)
          .reshape(E, CIN, KK, OTN, 128)
          .transpose(0, 1, 3, 2, 4))
    wt = np.ascontiguousarray(wt).astype(ml_dtypes.bfloat16)
    rwt = np.ascontiguousarray(route_w.T)  # [CIN, E]
    bnp = np.ascontiguousarray(
        np.stack([bn_gamma, bn_beta, bn_mean, bn_var], axis=1))  # [COUT, 4]

    # width-pad on host: border columns arrive pre-zeroed, so the device DMA
    # is one fully contiguous transfer per (sample, cin-tile)
    xpad = np.zeros((B, CIN, H, WP), dtype=ml_dtypes.bfloat16)
    xpad[:, :, :, 1:W + 1] = x.astype(ml_dtypes.bfloat16)

    return [
        {"x": np.ascontiguousarray(xpad[c * BPC:(c + 1) * BPC]),
         "wt": wt, "rwt": rwt, "rb": route_b, "bnp": bnp}
        for c in range(N_CORES)
    ]


def _get_nc():
    with _LOCK:
        if "nc" not in _CACHE:
            _CACHE["nc"] = build_bass()
        return _CACHE["nc"]


def kernel(**inputs):
    in_maps = prepare_in_maps(inputs)
    nc = _get_nc()
    res = run_bass_kernel_spmd(nc, in_maps, core_ids=list(range(N_CORES)))
    return np.concatenate([r["y"] for r in res.results], axis=0)


# revision 34
# speedup vs baseline: 1.4726x; 1.0222x over previous
"""CondConv (routing -> per-sample mixed 3x3 conv -> frozen BN -> ReLU -> residual)
on 8 Trainium2 NeuronCores, data-parallel over batch (4 samples/core).

Per core:
  - expert bank resident in SBUF as bf16, host-pretransposed to
    [cin, cout-half, kk, 128] so each cout half is contiguous
  - routing: GAP (DVE reduce) -> dot with route_w (DVE + gpsimd partition
    all-reduce; keeps the PE queue free for conv matmuls) -> sigmoid (ACT)
  - per-sample mixed kernel: DVE scalar_tensor_tensor accumulation in bf16,
    split per cout half so the first conv starts after half the mixing
  - conv: per output tile, 18 accumulating bf16 matmuls (2 cin tiles x 3x3
    taps; fp32 PSUM) against width-padded bf16 images; moving dim = 8 rows
    x 56 cols = 448; bf16 weight loads get FWL so LDW hides under the stream
  - BN(frozen)+ReLU fused into the ACT PSUM evacuation, residual add on DVE,
    fp32 output
"""

import threading

import ml_dtypes
import numpy as np

import concourse.bass as bass
import concourse.mybir as mybir
import concourse.tile as tile
from concourse import bacc, bass_isa
from concourse.bass_utils import run_bass_kernel_spmd

F32 = mybir.dt.float32
BF16 = mybir.dt.bfloat16
AX = mybir.AxisListType
OP = mybir.AluOpType
AF = mybir.ActivationFunctionType

N_CORES = 8
B, CIN, COUT, H, W, KS, E = 32, 256, 256, 56, 56, 3, 4
BPC = B // N_CORES  # samples per core
CT = CIN // 128     # cin partition tiles
OTN = COUT // 128   # cout partition tiles
KK = KS * KS
WP = W + 2          # width zero-padded (kj shifts); height handled by clipping
RC = 7              # row chunks per image
RH = H // RC        # rows per chunk
NF = RH * W         # moving-dim elements per matmul
BN_EPS = 1e-5

# conv taps, center first: the center tap covers the full output chunk, so it
# carries start=True and clears every PSUM has_written bit; row-clipped taps
# then accumulate flat sub-slices (= 'same' padding semantics at top/bottom).
TAPS = [(1, 1)] + [(ki, kj) for ki in range(KS) for kj in range(KS)
                   if (ki, kj) != (1, 1)]


def build_bass():
    nc = bacc.Bacc("TRN2", target_bir_lowering=False, debug=False)

    x_d = nc.dram_tensor("x", [BPC, CIN, H, WP], BF16, kind="ExternalInput")
    wt_d = nc.dram_tensor("wt", [E, CIN, OTN, KK, 128], BF16,
                          kind="ExternalInput")
    rwt_d = nc.dram_tensor("rwt", [CIN, E], F32, kind="ExternalInput")
    rb_d = nc.dram_tensor("rb", [E], F32, kind="ExternalInput")
    bnp_d = nc.dram_tensor("bnp", [COUT, 4], F32, kind="ExternalInput")
    y_d = nc.dram_tensor("y", [BPC, COUT, H, W], F32, kind="ExternalOutput")

    x_ap = x_d.ap()
    wt_ap = wt_d.ap()
    rwt_ap = rwt_d.ap()
    rb_ap = rb_d.ap()
    bnp_ap = bnp_d.ap()
    y_ap = y_d.ap()

    with tile.TileContext(nc) as tc:
        with (
            tc.tile_pool(name="wbp", bufs=1) as wbp,
            tc.tile_pool(name="xpp", bufs=1) as xpp,
            tc.tile_pool(name="mwp", bufs=1) as mwp,
            tc.tile_pool(name="otp", bufs=10) as otp,
            tc.tile_pool(name="snp", bufs=1) as snp,
            tc.tile_pool(name="smp", bufs=4) as smp,
            tc.tile_pool(name="psp", bufs=6, space="PSUM") as psp,
        ):
            # ---- persistent tiles ----
            wb = [[wbp.tile([128, OTN, KK, 128], BF16, name=f"wb{e}_{t}",
                            tag=f"wb{e}_{t}")
                   for t in range(CT)] for e in range(E)]
            xp = [[xpp.tile([128, H, WP], BF16, name=f"xp{i}_{t}",
                            tag=f"xp{i}_{t}")
                   for t in range(CT)] for i in range(2)]
            mw = [[mwp.tile([128, OTN, KK, 128], BF16, name=f"mw{i}_{t}",
                            tag=f"mw{i}_{t}")
                   for t in range(CT)] for i in range(2)]
            rwt_sb = [snp.tile([128, E], F32, name=f"rwt{t}", tag=f"rwt{t}")
                      for t in range(CT)]
            rb_bc = snp.tile([128, E], F32, name="rb_bc", tag="rb_bc")
            bn_sb = [snp.tile([128, 4], F32, name=f"bn{o}", tag=f"bn{o}")
                     for o in range(OTN)]
            bn_inv = [snp.tile([128, 1], F32, name=f"bninv{o}", tag=f"bninv{o}")
                      for o in range(OTN)]
            bn_shift = [snp.tile([128, 1], F32, name=f"bnsh{o}", tag=f"bnsh{o}")
                        for o in range(OTN)]

            # ---- preamble DMAs in priority order: queue order = bandwidth
            # priority. x(0) tiles split across two queues (routing critical
            # path), tiny params next, then the expert bank with the oi=0
            # halves first (mixing consumes them first).
            nc.sync.dma_start(out=xp[0][0], in_=x_ap[0, 0:128, :, :])
            nc.scalar.dma_start(out=xp[0][1], in_=x_ap[0, 128:256, :, :])
            for t in range(CT):
                nc.sync.dma_start(out=rwt_sb[t],
                                  in_=rwt_ap[t * 128:(t + 1) * 128, :])
            nc.sync.dma_start(
                out=rb_bc,
                in_=bass.AP(tensor=rb_ap.tensor, offset=0, ap=[[0, 128], [1, E]]))
            for o in range(OTN):
                nc.sync.dma_start(out=bn_sb[o],
                                  in_=bnp_ap[o * 128:(o + 1) * 128, :])
            for oi in range(OTN):
                for e in range(E):
                    for t in range(CT):
                        nc.sync.dma_start(out=wb[e][t][:, oi],
                                          in_=wt_ap[e, t * 128:(t + 1) * 128, oi])

            # all-zeros per-partition scalar: explicit AP bias for ACT funcs
            # (the float-bias path needs a pre-registered const-AP database)
            zeros1 = snp.tile([128, 1], F32, name="zeros1", tag="zeros1")
            nc.vector.memset(zeros1, 0.0)

            # BN folded scale/shift: inv = gamma / sqrt(var+eps);
            # shift = beta - mean * inv
            for o in range(OTN):
                ve = smp.tile([128, 1], F32, name=f"ve{o}", tag="ve")
                nc.vector.tensor_scalar_add(ve, bn_sb[o][:, 3:4], BN_EPS)
                sq = smp.tile([128, 1], F32, name=f"sq{o}", tag="sq")
                nc.scalar.activation(out=sq, in_=ve, func=AF.Sqrt, bias=zeros1)
                nc.vector.reciprocal(out=bn_inv[o], in_=sq)
                nc.vector.tensor_mul(bn_inv[o], bn_inv[o], bn_sb[o][:, 0:1])
                mi = smp.tile([128, 1], F32, name=f"mi{o}", tag="mi")
                nc.vector.tensor_mul(mi, bn_sb[o][:, 2:3], bn_inv[o])
                nc.vector.tensor_sub(bn_shift[o], bn_sb[o][:, 1:2], mi)

            # scratch target for the ACT-side pooled copy (only accum_out used)
            pscr = snp.tile([128, H * W], BF16, name="pscr", tag="pscr")

            def prep(s):
                """Routing + weight mixing for sample s (no PE involvement)."""
                i = s % 2
                pooled = [smp.tile([128, 1], F32, name=f"pool{s}_{t}",
                                   tag=f"pool{t}") for t in range(CT)]
                # GAP: tile 0 on DVE, tile 1 on ACT (Copy + accum_out) so the
                # two image reductions run in parallel
                nc.vector.reduce_sum(out=pooled[0], in_=xp[i][0][:, :, 1:W + 1],
                                     axis=AX.XY)
                nc.scalar.activation(out=pscr, in_=xp[i][1][:, :, 1:W + 1],
                                     func=AF.Copy, accum_out=pooled[1])
                prod = smp.tile([128, E], F32, name=f"prod{s}", tag="prod")
                nc.vector.tensor_scalar_mul(prod, rwt_sb[0], pooled[0])
                nc.vector.scalar_tensor_tensor(out=prod, in0=rwt_sb[1],
                                               scalar=pooled[1], in1=prod,
                                               op0=OP.mult, op1=OP.add)
                lg = smp.tile([128, E], F32, name=f"lg{s}", tag="lg")
                nc.gpsimd.partition_all_reduce(lg, prod, channels=128,
                                               reduce_op=bass_isa.ReduceOp.add)
                nc.vector.scalar_tensor_tensor(out=lg, in0=lg,
                                               scalar=1.0 / (H * W), in1=rb_bc,
                                               op0=OP.mult, op1=OP.add)
                rr = smp.tile([128, E], F32, name=f"rr{s}", tag="rr")
                nc.scalar.activation(out=rr, in_=lg, func=AF.Sigmoid, bias=zeros1)
                # mix per cout half: the first conv of the sample only waits
                # for the oi=0 half of the bank. cin tile 0 accumulates on
                # DVE; tile 1 gets its expert scaling from ACT (scaled Copy)
                # with DVE doing only the adds, so the two chains overlap.
                for oi in range(OTN):
                    nc.vector.tensor_scalar_mul(mw[i][0][:, oi],
                                                wb[0][0][:, oi], rr[:, 0:1])
                    for e in range(1, E):
                        nc.vector.scalar_tensor_tensor(
                            out=mw[i][0][:, oi], in0=wb[e][0][:, oi],
                            scalar=rr[:, e:e + 1], in1=mw[i][0][:, oi],
                            op0=OP.mult, op1=OP.add)
                    ce = [smp.tile([128, KK, 128], BF16, name=f"ce{s}_{oi}_{e}",
                                   tag=f"ce{e}", bufs=2) for e in range(E)]
                    for e in range(E):
                        nc.scalar.activation(out=ce[e], in_=wb[e][1][:, oi],
                                             func=AF.Copy, scale=rr[:, e:e + 1])
                    nc.vector.tensor_add(mw[i][1][:, oi], ce[0], ce[1])
                    nc.vector.tensor_add(mw[i][1][:, oi], mw[i][1][:, oi], ce[2])
                    nc.vector.tensor_add(mw[i][1][:, oi], mw[i][1][:, oi], ce[3])

            def conv(s, oi):
                """One output channel tile of sample s: matmuls + BN/ReLU +
                residual + store."""
                i = s % 2
                o0 = oi * 128
                n_mm = len(TAPS) * CT
                for rc in range(RC):
                    r0 = rc * RH
                    acc = psp.tile([128, NF], F32, name=f"acc{s}_{oi}_{rc}",
                                   tag="acc")
                    k = 0
                    for ki, kj in TAPS:
                        # valid output rows for this tap (clipped at top/
                        # bottom; kj handled by the zero-padded columns)
                        h_lo = max(r0, 1 - ki)
                        h_hi = min(r0 + RH - 1, H - ki)
                        kki = ki * KS + kj
                        for t in range(CT):
                            nc.tensor.matmul(
                                acc[:, (h_lo - r0) * W:(h_hi - r0 + 1) * W],
                                lhsT=mw[i][t][:, oi, kki, :],
                                rhs=xp[i][t][:, h_lo + ki - 1:h_hi + ki,
                                             kj:kj + W],
                                start=(k == 0), stop=(k == n_mm - 1))
                            k += 1
                    ob = otp.tile([128, NF], F32, name=f"ob{s}_{oi}_{rc}",
                                  tag="ob")
                    nc.scalar.activation(out=ob[:], in_=acc[:], func=AF.Relu,
                                         bias=bn_shift[oi], scale=bn_inv[oi])
                    ob3 = ob.rearrange("p (a b) -> p a b", a=RH)
                    nc.vector.tensor_add(ob3, ob3,
                                         xp[i][oi][:, r0:r0 + RH, 1:W + 1])
                    nc.sync.dma_start(out=y_ap[s, o0:o0 + 128, r0:r0 + RH, :],
                                      in_=ob3)

            prep(0)
            for s in range(BPC):
                if s + 1 < BPC:
                    j = (s + 1) % 2
                    for t in range(CT):
                        nc.sync.dma_start(
                            out=xp[j][t],
                            in_=x_ap[s + 1, t * 128:(t + 1) * 128, :, :])
                conv(s, 0)
                if s + 1 < BPC:
                    prep(s + 1)
                conv(s, 1)

    nc.compile()
    return nc


_CACHE = {}
_LOCK = threading.Lock()


def prepare_in_maps(inputs):
    """Host-side layout prep (sharding + transposes + dtype casts only)."""
    x = np.asarray(inputs["x"], dtype=np.float32)
    route_w = np.asarray(inputs["route_w"], dtype=np.float32)
    route_b = np.ascontiguousarray(np.asarray(inputs["route_b"], dtype=np.float32))
    expert_w = np.asarray(inputs["expert_w"], dtype=np.float32)
    bn_gamma = np.asarray(inputs["bn_gamma"], dtype=np.float32)
    bn_beta = np.asarray(inputs["bn_beta"], dtype=np.float32)
    bn_mean = np.asarray(inputs["bn_mean"], dtype=np.float32)
    bn_var = np.asarray(inputs["bn_var"], dtype=np.float32)

    # [E, COUT, CIN, K, K] -> [E, CIN, K, K, COUT] -> [E, CIN, OTN, KK, 128]
    wt = (expert_w.transpose(0, 2, 3, 4, 1)
          .reshape(E, CIN, KK, OTN, 128)
          .transpose(0, 1, 3, 2, 4))
    wt = np.ascontiguousarray(wt).astype(ml_dtypes.bfloat16)
    rwt = np.ascontiguousarray(route_w.T)  # [CIN, E]
    bnp = np.ascontiguousarray(
        np.stack([bn_gamma, bn_beta, bn_mean, bn_var], axis=1))  # [COUT, 4]

    # width-pad on host: border columns arrive pre-zeroed, so the device DMA
    # is one fully contiguous transfer per (sample, cin-tile)
    xpad = np.zeros((B, CIN, H, WP), dtype=ml_dtypes.bfloat16)
    xpad[:, :, :, 1:W + 1] = x.astype(ml_dtypes.bfloat16)

    return [
        {"x": np.ascontiguousarray(xpad[c * BPC:(c + 1) * BPC]),
         "wt": wt, "rwt": rwt, "rb": route_b, "bnp": bnp}
        for c in range(N_CORES)
    ]


def _get_nc():
    with _LOCK:
        if "nc" not in _CACHE:
            _CACHE["nc"] = build_bass()
        return _CACHE["nc"]


def kernel(**inputs):
    in_maps = prepare_in_maps(inputs)
    nc = _get_nc()
    res = run_bass_kernel_spmd(nc, in_maps, core_ids=list(range(N_CORES)))
    return np.concatenate([r["y"] for r in res.results], axis=0)
